# revision 1
# baseline (speedup 1.0000x reference)
"""KV-cache sliding-window update for Trainium2 (Bass), 8-core SPMD.

Reference semantics (per batch b, head h):
    C = concat([cache, new], time)                  # [T + T_NEW]
    out = concat([C[:SINK], C[-WINDOW:]], time)     # [SINK + WINDOW]

With T=4096, T_NEW=16, WINDOW=4096, SINK=4 this is pure data movement:
    out[0:4]      = cache[0:4]        (sink tokens)
    out[4:4084]   = cache[16:4096]    (kept window, 4080 rows)
    out[4084:4100]= new[0:16]         (new tokens)

Each (b, h) row is independent, so we shard the flattened (B*H) = 128 rows
across 8 NeuronCores (16 rows each; equivalent to batch x head-half tensor
parallel). Per core the NEFF is just 6 DRAM->DRAM DMA copies (3 per K/V
tensor) issued on the HWDGE queue — no SBUF staging, no compute.
"""

import numpy as np

import concourse.bass as bass
import concourse.mybir as mybir
from concourse.bass_utils import run_bass_kernel_spmd

B, H, T, T_NEW, D = 4, 32, 4096, 16, 128
WINDOW, SINK = 4096, 4
T_OUT = SINK + WINDOW            # 4100
MID_START = T + T_NEW - WINDOW   # 16: first kept row of the old cache
MID = T - MID_START              # 4080 kept rows
N_CORES = 8
R = B * H                        # 128 independent (b, h) rows
R_LOC = R // N_CORES             # 16 rows per core

TRACE = False          # test.py flips this to capture an NTFF profile
LAST_RESULTS = None    # BassKernelResults of the most recent run (for test.py)

_NC = None


def _build_nc():
    nc = bass.Bass()
    f32 = mybir.dt.float32
    k = nc.dram_tensor("K", [R_LOC, T, D], f32, kind="ExternalInput")
    v = nc.dram_tensor("V", [R_LOC, T, D], f32, kind="ExternalInput")
    kn = nc.dram_tensor("K_new", [R_LOC, T_NEW, D], f32, kind="ExternalInput")
    vn = nc.dram_tensor("V_new", [R_LOC, T_NEW, D], f32, kind="ExternalInput")
    ko = nc.dram_tensor("K_out", [R_LOC, T_OUT, D], f32, kind="ExternalOutput")
    vo = nc.dram_tensor("V_out", [R_LOC, T_OUT, D], f32, kind="ExternalOutput")

    with nc.Block() as block, nc.semaphore("dma_sem") as sem:

        @block.sync
        def _(sync):
            n = 0
            for src, new, dst in ((k, kn, ko), (v, vn, vo)):
                # kept window: old rows [16, 4096) -> out rows [4, 4084)
                sync.dma_start(
                    dst[:, SINK : SINK + MID, :], src[:, MID_START:T, :]
                ).then_inc(sem, 16)
                # sink tokens: old rows [0, 4) -> out rows [0, 4)
                sync.dma_start(dst[:, 0:SINK, :], src[:, 0:SINK, :]).then_inc(sem, 16)
                # new tokens -> out rows [4084, 4100)
                sync.dma_start(
                    dst[:, SINK + MID : T_OUT, :], new[:, :, :]
                ).then_inc(sem, 16)
                n += 3
            sync.wait_ge(sem, 16 * n)

    return nc


def kernel(K, V, K_new, V_new):
    global _NC, LAST_RESULTS
    if _NC is None:
        _NC = _build_nc()

    ins = {
        "K": np.asarray(K, dtype=np.float32).reshape(R, T, D),
        "V": np.asarray(V, dtype=np.float32).reshape(R, T, D),
        "K_new": np.asarray(K_new, dtype=np.float32).reshape(R, T_NEW, D),
        "V_new": np.asarray(V_new, dtype=np.float32).reshape(R, T_NEW, D),
    }
    in_maps = [
        {name: arr[c * R_LOC : (c + 1) * R_LOC] for name, arr in ins.items()}
        for c in range(N_CORES)
    ]
    LAST_RESULTS = run_bass_kernel_spmd(
        _NC, in_maps, core_ids=list(range(N_CORES)), trace=TRACE
    )
    res = LAST_RESULTS.results
    K_out = np.concatenate([r["K_out"] for r in res], axis=0).reshape(B, H, T_OUT, D)
    V_out = np.concatenate([r["V_out"] for r in res], axis=0).reshape(B, H, T_OUT, D)
    return K_out, V_out


# revision 2
# speedup vs baseline: 1.3253x; 1.3253x over previous
"""KV-cache sliding-window update for Trainium2 (Bass), 8-core SPMD.

Reference semantics (per batch b, head h):
    C = concat([cache, new], time)                  # [T + T_NEW]
    out = concat([C[:SINK], C[-WINDOW:]], time)     # [SINK + WINDOW]

With T=4096, T_NEW=16, WINDOW=4096, SINK=4 this is pure data movement:
    out[0:4]      = cache[0:4]        (sink tokens)
    out[4:4084]   = cache[16:4096]    (kept window, 4080 rows)
    out[4084:4100]= new[0:16]         (new tokens)

Each (b, h) row is independent, so we shard the flattened (B*H) = 128 rows
across 8 NeuronCores (16 rows each; equivalent to batch x head-half tensor
parallel). Per core the NEFF is just 6 DRAM->DRAM DMA copies (3 per K/V
tensor) issued on the HWDGE queue — no SBUF staging, no compute.
"""

import numpy as np

import concourse.bass as bass
import concourse.mybir as mybir
from concourse.bass_utils import run_bass_kernel_spmd

B, H, T, T_NEW, D = 4, 32, 4096, 16, 128
WINDOW, SINK = 4096, 4
T_OUT = SINK + WINDOW            # 4100
MID_START = T + T_NEW - WINDOW   # 16: first kept row of the old cache
MID = T - MID_START              # 4080 kept rows
N_CORES = 8
R = B * H                        # 128 independent (b, h) rows
R_LOC = R // N_CORES             # 16 rows per core

TRACE = False          # test.py flips this to capture an NTFF profile
LAST_RESULTS = None    # BassKernelResults of the most recent run (for test.py)

_NC = None


def _build_nc():
    nc = bass.Bass()
    f32 = mybir.dt.float32
    k = nc.dram_tensor("K", [R_LOC, T, D], f32, kind="ExternalInput")
    v = nc.dram_tensor("V", [R_LOC, T, D], f32, kind="ExternalInput")
    kn = nc.dram_tensor("K_new", [R_LOC, T_NEW, D], f32, kind="ExternalInput")
    vn = nc.dram_tensor("V_new", [R_LOC, T_NEW, D], f32, kind="ExternalInput")
    ko = nc.dram_tensor("K_out", [R_LOC, T_OUT, D], f32, kind="ExternalOutput")
    vo = nc.dram_tensor("V_out", [R_LOC, T_OUT, D], f32, kind="ExternalOutput")

    with nc.Block() as block, nc.semaphore("dma_sem") as sem, nc.semaphore(
        "dma_sem2"
    ) as sem2:
        # K on the Sync HWDGE ring, V on the Scalar HWDGE ring: two queues so
        # each SDMA engine interleaves work from both and can overlap the
        # read/write turnaround of one queue's descriptor with the other's.

        @block.sync
        def _(sync):
            src, new, dst = k, kn, ko
            # kept window: old rows [16, 4096) -> out rows [4, 4084)
            sync.dma_start(
                dst[:, SINK : SINK + MID, :], src[:, MID_START:T, :]
            ).then_inc(sem, 16)
            # sink tokens: old rows [0, 4) -> out rows [0, 4)
            sync.dma_start(dst[:, 0:SINK, :], src[:, 0:SINK, :]).then_inc(sem, 16)
            # new tokens -> out rows [4084, 4100)
            sync.dma_start(dst[:, SINK + MID : T_OUT, :], new[:, :, :]).then_inc(
                sem, 16
            )
            sync.wait_ge(sem, 48)

        @block.scalar
        def _(scalar):
            src, new, dst = v, vn, vo
            scalar.dma_start(
                dst[:, SINK : SINK + MID, :], src[:, MID_START:T, :]
            ).then_inc(sem2, 16)
            scalar.dma_start(dst[:, 0:SINK, :], src[:, 0:SINK, :]).then_inc(sem2, 16)
            scalar.dma_start(dst[:, SINK + MID : T_OUT, :], new[:, :, :]).then_inc(
                sem2, 16
            )
            scalar.wait_ge(sem2, 48)

    return nc


def kernel(K, V, K_new, V_new):
    global _NC, LAST_RESULTS
    if _NC is None:
        _NC = _build_nc()

    ins = {
        "K": np.asarray(K, dtype=np.float32).reshape(R, T, D),
        "V": np.asarray(V, dtype=np.float32).reshape(R, T, D),
        "K_new": np.asarray(K_new, dtype=np.float32).reshape(R, T_NEW, D),
        "V_new": np.asarray(V_new, dtype=np.float32).reshape(R, T_NEW, D),
    }
    in_maps = [
        {name: arr[c * R_LOC : (c + 1) * R_LOC] for name, arr in ins.items()}
        for c in range(N_CORES)
    ]
    LAST_RESULTS = run_bass_kernel_spmd(
        _NC, in_maps, core_ids=list(range(N_CORES)), trace=TRACE
    )
    res = LAST_RESULTS.results
    K_out = np.concatenate([r["K_out"] for r in res], axis=0).reshape(B, H, T_OUT, D)
    V_out = np.concatenate([r["V_out"] for r in res], axis=0).reshape(B, H, T_OUT, D)
    return K_out, V_out
